# revision 31
# baseline (speedup 1.0000x reference)
"""Adaptive embedding (Transformer-XL wt103) on 8 trn2 NeuronCores.

Strategy: token-parallel across the 8 cores (2048 tokens each, no
collectives). The host sorts each core's tokens by id so each embedding
bucket becomes one contiguous segment, dealt round-robin across cores
for near-perfect load balance.

Weight preprocessing (host, input-independent):
 - Buckets 0 and 1 (d_emb 1024/256, dense projection to 1024) are folded
   into one pre-projected table pp01[v] = emb[v] @ projT * sqrt(d_proj)
   in bf16. On device those tokens are a pure gather -> output DMA.
 - Buckets 2 and 3 (d=64/16) are merged into one 128-column table: e2
   rows occupy cols 0:64, e3 rows cols 64:80 (zero-padded elsewhere),
   stacked so local id = global id - 40000 for both. One stacked
   projection [128, 1024] = [proj2T; proj3T; 0] makes every merged token
   the SAME K=128 matmul (the zero blocks kill the cross terms).

Device graph per core:
 - pp01 tokens: per-128-token-tile indirect SWDGE gathers (one
   descriptor per partition; offsets int32 in SBUF), shipped straight
   out in one batched DMA.
 - merged-bucket tokens: transpose-mode dma_gather (mlp-library SWDGE
   ucode), one call per 32768-row table range (int16 index limit). The
   transposed gather lands the embedding tiles DIRECTLY as matmul
   weights lhsT [128, tokens] in SBUF — no PE transpose, no psum->lhsT
   copy. Index blocks are [16, nk/16]-wrapped and REPLICATED across all
   eight 16-partition groups (the Q7 ucode reads a queue-dependent
   group). Matmuls accumulate in PSUM, f32->bf16 casts alternate over
   the vector/scalar engines, batched output DMAs ride the sync ring.

SWDGE descriptor generation (~9ns/descriptor, serial on GpSimd) is the
critical path; everything else hides under it. Output travels as bf16
(rel err ~3e-3, well under the 2e-2 gate). The host undoes the sort
permutation and widens to f32 on the way back.
"""

import os
import sys
import types

for _p in (
    "/root/.axon_site",
    "/root/.axon_site/_ro/trn_rl_repo",
    "/root/.axon_site/_ro/pypackages",
    "/opt/trn_rl_repo",
):
    if _p not in sys.path:
        sys.path.append(_p)

import numpy as np
import ml_dtypes

# antenv.axon_hooks shim: lets BASS_TRACE=1 profile runs work under axon.
try:
    import antenv.axon_hooks  # noqa: F401
except ImportError:
    _hooks = types.ModuleType("antenv.axon_hooks")
    _hooks._hook = None
    _hooks.set_axon_ntff_profile_hook = lambda h: setattr(_hooks, "_hook", h)
    _hooks.get_axon_ntff_profile_hook = lambda: _hooks._hook
    import antenv

    antenv.axon_hooks = _hooks
    sys.modules["antenv.axon_hooks"] = _hooks
    try:
        from trn_agent_boot.trn_boot import _ntff_profile_via_ctypes

        _h = _ntff_profile_via_ctypes("/opt/axon/libaxon_pjrt.so")
        if _h is not None:
            _hooks.set_axon_ntff_profile_hook(_h)
    except Exception:
        pass

import concourse.bacc as bacc
import concourse.bass as bass
import concourse.mybir as mybir
import concourse.tile as tile
from concourse.tile_rust import add_dep_helper
from concourse.bass_utils import run_bass_kernel_spmd

N_TOKEN = 267735
D_PROJ = 1024
EMB_SCALE = float(D_PROJ) ** 0.5
NCORES = 8
BF16 = ml_dtypes.bfloat16
D23 = 128  # merged bucket-2/3 row width (64 + 16, padded to 256B)
PP_HI = 40000  # pp01 covers global ids [0, 40000)
RW = 32768  # dma_gather int16 index range width

LAST_RESULT = None  # BassKernelResults of the most recent run (for test.py)


def _prepare(inp, emb0, emb1, emb2, emb3, proj0, proj1, proj2, proj3):
    """Host planning shared by kernel() and the sim harness."""
    ids = np.asarray(inp).reshape(-1).astype(np.int64)
    n_tok = ids.shape[0]
    assert n_tok % NCORES == 0

    # --- tables (input-independent weight preprocessing) ---
    pp0 = (emb0 @ proj0.T) * EMB_SCALE
    pp1 = (emb1 @ proj1.T) * EMB_SCALE
    pp01 = np.ascontiguousarray(np.concatenate([pp0, pp1], axis=0).astype(BF16))
    em23 = np.zeros((N_TOKEN - PP_HI, D23), BF16)
    em23[: emb2.shape[0], :64] = emb2.astype(BF16)
    em23[emb2.shape[0] :, 64:80] = emb3.astype(BF16)
    p23 = np.zeros((D23, D_PROJ), np.float32)
    p23[:64] = proj2.T * EMB_SCALE
    p23[64:80] = proj3.T * EMB_SCALE
    p23 = np.ascontiguousarray(p23.astype(BF16))

    # --- sort + deal round-robin ---
    order = np.argsort(ids, kind="stable")
    sids = ids[order]
    g_mid = np.searchsorted(sids, PP_HI, "left")

    # pp01 segment (direct): int32 idx, per-128-token-tile indirect gathers
    toks01 = order[:g_mid]
    locs01 = sids[:g_mid].astype(np.int32)
    counts = [len(locs01[c::NCORES]) for c in range(NCORES)]
    pp_live = max(counts)
    nt_pp = -(-pp_live // 128) * 128 // 128
    pp_toks = []
    idx32 = []
    for c in range(NCORES):
        li = locs01[c::NCORES]
        pad = np.zeros(nt_pp * 128, np.int32)
        pad[: len(li)] = li
        idx32.append(pad.reshape(nt_pp, 128).T)  # [128, nt_pp]
        pp_toks.append(toks01[c::NCORES])

    # em23 segment (mm): int16 idx per 32768-row range, transpose dma_gather
    toks23 = order[g_mid:]
    locs23 = (sids[g_mid:] - PP_HI).astype(np.int64)
    per_core_locs = [locs23[c::NCORES] for c in range(NCORES)]
    per_core_toks = [toks23[c::NCORES] for c in range(NCORES)]
    nrange = -(-(N_TOKEN - PP_HI) // RW)
    ranges = []  # (rk, nk, live, cb16, obase)
    mm_toks = []  # per range: [per-core token arrays]
    flat16 = [[] for _ in range(NCORES)]
    cb16 = 0
    obase = 0
    for k in range(nrange):
        lo, hi = k * RW, min((k + 1) * RW, N_TOKEN - PP_HI)
        seg_locs = []
        seg_toks = []
        for c in range(NCORES):
            a = np.searchsorted(per_core_locs[c], lo, "left")
            b = np.searchsorted(per_core_locs[c], hi, "left")
            seg_locs.append((per_core_locs[c][a:b] - lo).astype(np.int16))
            seg_toks.append(per_core_toks[c][a:b])
        live = max(len(s) for s in seg_locs)
        if live == 0:
            continue
        nk = -(-live // 128) * 128
        for c in range(NCORES):
            pad = np.zeros(nk, np.int16)
            pad[: len(seg_locs[c])] = seg_locs[c]
            flat16[c].append(pad)
        ranges.append({"rk": k, "nk": nk, "live": live, "cb16": cb16, "obase": obase})
        mm_toks.append(seg_toks)
        cb16 += nk // 16
        obase += nk // 128
    s16 = cb16
    nt_mm = obase

    in_maps = []
    for c in range(NCORES):
        flat = np.concatenate(flat16[c])
        # [16, S] wrap (element (p,s) = flat[s*16+p]), replicated across all
        # eight 16-partition groups for the Q7 ucode
        idx16 = np.ascontiguousarray(
            np.tile(flat.reshape(s16, 16).T, (8, 1))
        )
        in_maps.append(
            {
                "pp01": pp01,
                "em23": em23,
                "projt": p23,
                "idx32": np.ascontiguousarray(idx32[c]),
                "idx16": idx16,
            }
        )

    meta = {
        "nt_pp": nt_pp,
        "pp_live": pp_live,
        "ranges": ranges,
        "s16": s16,
        "nt_mm": nt_mm,
        "pp_toks": pp_toks,
        "mm_toks": mm_toks,
        "n_tok": n_tok,
    }
    return in_maps, meta


def _build_graph(meta):
    nc = bacc.Bacc(None, target_bir_lowering=False, debug=False)
    dt = mybir.dt
    nt_pp, ranges, s16, nt_mm = (
        meta["nt_pp"], meta["ranges"], meta["s16"], meta["nt_mm"],
    )

    pp_par = nc.declare_dram_parameter("pp01", [PP_HI, D_PROJ], dt.bfloat16, False)
    em_par = nc.declare_dram_parameter(
        "em23", [N_TOKEN - PP_HI, D23], dt.bfloat16, False
    )
    proj_par = nc.declare_dram_parameter("projt", [D23, D_PROJ], dt.bfloat16, False)
    ix32_par = nc.declare_dram_parameter("idx32", [128, nt_pp], dt.int32, False)
    ix16_par = nc.declare_dram_parameter("idx16", [128, s16], dt.int16, False)
    outmm_par = nc.declare_dram_parameter(
        "outmm", [128, nt_mm, D_PROJ], dt.bfloat16, True
    )
    outpp_par = nc.declare_dram_parameter(
        "outpp", [128, nt_pp, D_PROJ], dt.bfloat16, True
    )

    # raw (non-pool) SBUF tensors for the gather side; consumers get
    # explicit add_dep_helper edges (the tile framework does not track
    # raw tensors), and only the first gather of each index dtype waits
    # on its index load — the rest follow via Pool program order.
    ix32_sb = nc.alloc_sbuf_tensor("ix32sb", [128, nt_pp], dt.int32)
    ix16_sb = nc.alloc_sbuf_tensor("ix16sb", [128, s16], dt.int16)
    etpp_sb = nc.alloc_sbuf_tensor("etpp", [128, nt_pp * D_PROJ], dt.bfloat16)
    dgt_sb = {
        r["rk"]: nc.alloc_sbuf_tensor(f"dgt{r['rk']}", [128, 1, r["nk"]], dt.bfloat16)
        for r in ranges
    }

    with tile.TileContext(nc) as tc:
        with (
            tc.tile_pool(name="const", bufs=1) as cpool,
            tc.tile_pool(name="outs", bufs=6) as opool,
            tc.tile_pool(name="ps", bufs=6, space="PSUM") as ppool,
        ):
            # index loads first on the sync HWDGE ring, then the projection
            ixdma32 = nc.sync.dma_start(ix32_sb[:], ix32_par[:])
            ixdma16 = nc.sync.dma_start(ix16_sb[:], ix16_par[:])
            proj_sb = cpool.tile([D23, D_PROJ], dt.bfloat16, tag="proj")
            nc.sync.dma_start(proj_sb[:], proj_par[:])

            # pp01 indirect gathers (one per 128-token tile, one descriptor
            # per partition), FIRST so the big pass-through output flows
            # through still-empty DMA queues early
            pp_g = []
            for t in range(nt_pp):
                gi = nc.gpsimd.indirect_dma_start(
                    out=etpp_sb[:, t * D_PROJ : (t + 1) * D_PROJ],
                    out_offset=None,
                    in_=pp_par[:],
                    in_offset=bass.IndirectOffsetOnAxis(
                        ap=ix32_sb[:, t : t + 1], axis=0
                    ),
                )
                pp_g.append(gi)
            add_dep_helper(pp_g[0].ins, ixdma32.ins, reason="gather reads idx32")

            # em23 transpose-mode dma_gather: one call per 32768-row range;
            # lands lhsT [128, nk] directly in SBUF
            mm_g = {}
            for i, r in enumerate(ranges):
                rk, nk = r["rk"], r["nk"]
                lo = rk * RW
                hi = min(lo + RW, N_TOKEN - PP_HI)
                gi = nc.gpsimd.dma_gather(
                    dgt_sb[rk][:],
                    em_par[lo:hi, :],
                    ix16_sb[:, r["cb16"] : r["cb16"] + nk // 16],
                    nk,
                    nk,
                    D23,
                    transpose=True,
                )
                mm_g[rk] = gi
                if i == 0:
                    add_dep_helper(gi.ins, ixdma16.ins, reason="gather reads idx16")

            # pp01 batched pass-through out (sync ring, issued the moment its
            # gathers land; garbage in padding slots is fine — the host only
            # reads live rows)
            od = nc.sync.dma_start(
                outpp_par[:, :nt_pp, :], etpp_sb[:, : nt_pp * D_PROJ]
            )
            for gi in pp_g:
                add_dep_helper(od.ins, gi.ins, reason="out reads gathered rows")

            # mm compute: per 128-token tile, two matmuls straight off the
            # transposed gather -> two psum->bf16 casts (both casts of tile t
            # on engine t%2) -> batched pair out DMA on the sync ring
            engs = [nc.vector, nc.scalar]
            tglob = 0
            for r in ranges:
                rk, nk, live = r["rk"], r["nk"], r["live"]
                nt = nk // 128
                nrow_last = (live - 1) % 128 + 1
                for tb in range(0, nt, 2):
                    gsz = min(2, nt - tb)
                    out_sb = opool.tile(
                        [128, 2, D_PROJ], dt.bfloat16, tag="osb", name="osb"
                    )
                    for ti in range(gsz):
                        t = tb + ti
                        lhsT = dgt_sb[rk][:, 0, t * 128 : (t + 1) * 128]
                        ceng = engs[tglob % 2]
                        tglob += 1
                        for nh in range(2):
                            ps = ppool.tile(
                                [128, 512], dt.float32, tag="ps", name="ps"
                            )
                            mm = nc.tensor.matmul(
                                ps[:],
                                lhsT,
                                proj_sb[:, nh * 512 : (nh + 1) * 512],
                                start=True,
                                stop=True,
                            )
                            add_dep_helper(
                                mm.ins, mm_g[rk].ins,
                                reason="matmul reads transposed gather",
                            )
                            dst = out_sb[:, ti, nh * 512 : (nh + 1) * 512]
                            if ceng is nc.vector:
                                ceng.tensor_copy(dst, ps[:])
                            else:
                                ceng.copy(dst, ps[:])
                    t0 = r["obase"] + tb
                    has_partial = (tb + gsz == nt) and nrow_last < 128
                    nfull = gsz - 1 if has_partial else gsz
                    if nfull:
                        nc.sync.dma_start(
                            outmm_par[:, t0 : t0 + nfull, :], out_sb[:, :nfull, :]
                        )
                    if has_partial:
                        nc.sync.dma_start(
                            outmm_par[:nrow_last, t0 + nfull, :],
                            out_sb[:nrow_last, nfull, :],
                        )

    nc.compile()
    return nc


def _unshard(meta, results):
    full = np.empty((meta["n_tok"], D_PROJ), np.float32)
    for c in range(NCORES):
        pp_rows = (
            np.asarray(results[c]["outpp"]).transpose(1, 0, 2).reshape(-1, D_PROJ)
        )
        mm_rows = (
            np.asarray(results[c]["outmm"]).transpose(1, 0, 2).reshape(-1, D_PROJ)
        )
        toks = meta["pp_toks"][c]
        if len(toks):
            full[toks] = pp_rows[: len(toks)]
        for r, seg_toks in zip(meta["ranges"], meta["mm_toks"]):
            toks = seg_toks[c]
            if len(toks):
                b = r["obase"] * 128
                full[toks] = mm_rows[b : b + len(toks)]
    return full


def kernel(inp, emb0, emb1, emb2, emb3, proj0, proj1, proj2, proj3):
    global LAST_RESULT
    in_maps, meta = _prepare(
        inp, np.asarray(emb0), np.asarray(emb1), np.asarray(emb2),
        np.asarray(emb3), np.asarray(proj0), np.asarray(proj1),
        np.asarray(proj2), np.asarray(proj3),
    )
    nc = _build_graph(meta)
    res = run_bass_kernel_spmd(nc, in_maps, core_ids=list(range(NCORES)))
    LAST_RESULT = res
    full = _unshard(meta, res.results)
    B, S = np.asarray(inp).shape
    return full.reshape(B, S, D_PROJ)


# revision 32
# speedup vs baseline: 1.3720x; 1.3720x over previous
"""Adaptive embedding (Transformer-XL wt103) on 8 trn2 NeuronCores.

Strategy: token-parallel across the 8 cores (2048 tokens each, no
collectives). The host sorts each core's tokens by id so each embedding
bucket becomes one contiguous segment, dealt round-robin across cores
for near-perfect load balance.

Weight preprocessing (host, input-independent):
 - Buckets 0 and 1 (d_emb 1024/256, dense projection to 1024) are folded
   into one pre-projected table pp01[v] = emb[v] @ projT * sqrt(d_proj)
   in bf16. On device those tokens are a pure gather -> output DMA.
 - Buckets 2 and 3 (d=64/16) are merged into one 80-column table: e2
   rows occupy cols 0:64, e3 rows cols 64:80 (zero-padded elsewhere),
   stacked so local id = global id - 40000 for both. One stacked
   projection [80, 1024] = [proj2T; proj3T] makes every merged token the
   SAME matmul (the zero blocks kill the cross terms). This costs one
   fused gather+matmul pipeline instead of two and one fewer 128-token
   gather call.

Device graph per core: per 128-token tile, an offset-driven indirect
DMA gather (one row per partition; the SWDGE ucode supports exactly 128
descriptors per call, ~1.4us of serial GpSimd time each — the critical
path). Merged-bucket tiles run transpose (PE) -> psum copy -> matmul
(K=80) -> bf16 output staging; pp01 tiles ship straight out in a single
batched output DMA. Output travels as bf16 (rel err ~3e-3, well under
the 2e-2 gate), halving the dominant output traffic. The host undoes
the sort permutation and widens to f32 on the way back.
"""

import os
import sys
import types

for _p in (
    "/root/.axon_site",
    "/root/.axon_site/_ro/trn_rl_repo",
    "/root/.axon_site/_ro/pypackages",
    "/opt/trn_rl_repo",
):
    if _p not in sys.path:
        sys.path.append(_p)

import numpy as np
import ml_dtypes

# antenv.axon_hooks shim: lets BASS_TRACE=1 profile runs work under axon.
try:
    import antenv.axon_hooks  # noqa: F401
except ImportError:
    _hooks = types.ModuleType("antenv.axon_hooks")
    _hooks._hook = None
    _hooks.set_axon_ntff_profile_hook = lambda h: setattr(_hooks, "_hook", h)
    _hooks.get_axon_ntff_profile_hook = lambda: _hooks._hook
    import antenv

    antenv.axon_hooks = _hooks
    sys.modules["antenv.axon_hooks"] = _hooks
    try:
        from trn_agent_boot.trn_boot import _ntff_profile_via_ctypes

        _h = _ntff_profile_via_ctypes("/opt/axon/libaxon_pjrt.so")
        if _h is not None:
            _hooks.set_axon_ntff_profile_hook(_h)
    except Exception:
        pass

import concourse.bacc as bacc
import concourse.bass as bass
import concourse.mybir as mybir
import concourse.tile as tile
from concourse.tile_rust import add_dep_helper
from concourse.bass_utils import run_bass_kernel_spmd

N_TOKEN = 267735
D_PROJ = 1024
EMB_SCALE = float(D_PROJ) ** 0.5
NCORES = 8
BF16 = ml_dtypes.bfloat16
D23 = 80  # merged bucket-2/3 row width (64 + 16)

# two segments: pre-projected buckets 0+1 (direct), merged buckets 2+3
SEGS = [
    {"name": "pp01", "lo": 0, "hi": 40000, "d": D_PROJ, "kind": "direct"},
    {"name": "em23", "lo": 40000, "hi": N_TOKEN, "d": D23, "kind": "mm"},
]
# gather order: the compute-free pp01 pass-through FIRST — its large
# batched output DMA then flows through empty queues early instead of
# crawling at the kernel tail; the merged mm segment follows and its
# last-tile compute chain forms the (shorter) tail
SEG_ORDER = [0, 1]

LAST_RESULT = None  # BassKernelResults of the most recent run (for test.py)


def _build_graph(plan, nt_total, s_pad):
    """plan: per active segment (in gather order) a dict with
    si, nt, n_live, cb (idx col base), slot (output slot base)."""
    nc = bacc.Bacc(None, target_bir_lowering=False, debug=False)
    dt = mybir.dt

    tab_par = {}
    for p in plan:
        s = SEGS[p["si"]]
        tab_par[p["si"]] = nc.declare_dram_parameter(
            s["name"], [s["hi"] - s["lo"], s["d"]], dt.bfloat16, False
        )
    proj_par = nc.declare_dram_parameter("projt", [D23, D_PROJ], dt.bfloat16, False)
    ident_par = nc.declare_dram_parameter("ident", [128, 128], dt.bfloat16, False)
    idx_par = nc.declare_dram_parameter("idxs", [128, nt_total], dt.int32, False)
    # partition-major outputs: slot s lives at [s % 128, s // 128, :].
    # Matmul-segment tiles ship straight from PSUM as f32 (no cast stage);
    # pp01 tiles pass through as bf16.
    nt_mm = sum(p["nt"] for p in plan if SEGS[p["si"]]["kind"] == "mm")
    nt_pp = sum(p["nt"] for p in plan if SEGS[p["si"]]["kind"] == "direct")
    outmm_par = nc.declare_dram_parameter(
        "outmm", [128, max(nt_mm, 1), D_PROJ], dt.bfloat16, True
    )
    outpp_par = nc.declare_dram_parameter(
        "outpp", [128, max(nt_pp, 1), D_PROJ], dt.bfloat16, True
    )

    # raw (non-pool) SBUF tensors for the gather side: only the FIRST
    # gather carries an explicit dep on the idx load; the rest follow via
    # Pool-engine program order. Consumers get explicit add_dep_helper
    # edges (the tile framework does not track raw tensors).
    idx_sb = nc.alloc_sbuf_tensor("idxsb", [128, nt_total], dt.int32)
    et_sb = {}
    for p in plan:
        si, nt = p["si"], p["nt"]
        d = SEGS[si]["d"]
        et_sb[si] = nc.alloc_sbuf_tensor(f"etsb{si}", [128, nt * d], dt.bfloat16)

    with tile.TileContext(nc) as tc:
        with (
            tc.tile_pool(name="const", bufs=1) as cpool,
            tc.tile_pool(name="lhsT", bufs=8) as ltpool,
            tc.tile_pool(name="outs", bufs=6) as opool,
            tc.tile_pool(name="ps", bufs=5, space="PSUM") as ppool,
            tc.tile_pool(name="ptr", bufs=3, space="PSUM") as trpool,
        ):
            # idx first on the sync HWDGE ring, then the small constants
            ixdma = nc.sync.dma_start(idx_sb[:], idx_par[:])
            ident = cpool.tile([128, 128], dt.bfloat16, tag="ident")
            nc.sync.dma_start(ident[:], ident_par[:])
            proj_sb = cpool.tile([D23, D_PROJ], dt.bfloat16, tag="proj")
            nc.sync.dma_start(proj_sb[:], proj_par[:])

            # indirect gathers, one per 128-token tile (the SWDGE ucode
            # generates one descriptor per partition: row idx_sb[p, col] of
            # the table lands in partition p; multi-column offset APs are NOT
            # supported by the hardware ucode, and offsets must live in SBUF).
            # The ~1.4us/call serial GpSimd time is the kernel's critical path.
            gather_insts = {}
            first_gather = None
            for p in plan:
                si, nt = p["si"], p["nt"]
                d = SEGS[si]["d"]
                for t in range(nt):
                    gi = nc.gpsimd.indirect_dma_start(
                        out=et_sb[si][:, t * d : (t + 1) * d],
                        out_offset=None,
                        in_=tab_par[si][:],
                        in_offset=bass.IndirectOffsetOnAxis(
                            ap=idx_sb[:, p["cb"] + t : p["cb"] + t + 1], axis=0
                        ),
                    )
                    gather_insts[(si, t)] = gi
                    if first_gather is None:
                        first_gather = gi
                        add_dep_helper(
                            gi.ins, ixdma.ins, reason="first gather reads idxs"
                        )

            # direct (pre-projected) segment first: gathered rows ARE output
            # rows. One batched full-tile DMA on the sync ring, emitted ahead
            # of the mm outs so it issues the moment its (early) gathers land
            # and its descriptors flow through still-empty queues. Garbage in
            # padding slots is fine — the host only reads live rows.
            for p in plan:
                si, nt, n_live = p["si"], p["nt"], p["n_live"]
                if SEGS[si]["kind"] != "direct":
                    continue
                t0 = p["obase"]
                od = nc.sync.dma_start(
                    outpp_par[:, t0 : t0 + nt, :], et_sb[si][:, : nt * D_PROJ]
                )
                for t in range(nt):
                    add_dep_helper(
                        od.ins, gather_insts[(si, t)].ins,
                        reason="out reads gathered rows",
                    )

            # merged mm segment: per tile, transpose (PE) -> lhsT copy ->
            # two matmuls -> two psum->bf16 casts -> batched out DMA on the
            # sync ring (SP does nothing else after the preamble). Engine
            # schedule avoids ping-pong bubbles: tile t's BOTH casts run on
            # engine t%2 while its lhsT copy runs on the OTHER engine.
            engs = [nc.vector, nc.scalar]
            for p in plan:
                si, nt, n_live = p["si"], p["nt"], p["n_live"]
                seg = SEGS[si]
                if seg["kind"] != "mm":
                    continue
                d = seg["d"]
                nrow_last = (n_live - 1) % 128 + 1
                for tb in range(0, nt, 2):
                    gsz = min(2, nt - tb)
                    out_sb = opool.tile(
                        [128, 2, D_PROJ], dt.bfloat16, tag="osb", name="osb"
                    )
                    for ti in range(gsz):
                        t = tb + ti
                        ptr = trpool.tile([d, 128], dt.bfloat16, tag="ptr", name="ptr")
                        tr = nc.tensor.transpose(
                            ptr[:], et_sb[si][:, t * d : (t + 1) * d], ident[:]
                        )
                        add_dep_helper(
                            tr.ins, gather_insts[(si, t)].ins,
                            reason="transpose reads gathered tile",
                        )
                        lhsT = ltpool.tile([d, 128], dt.bfloat16, tag="lt", name="lt")
                        ceng = engs[t % 2]
                        oeng = engs[(t + 1) % 2]
                        if oeng is nc.vector:
                            oeng.tensor_copy(lhsT[:], ptr[:])
                        else:
                            oeng.copy(lhsT[:], ptr[:])
                        for nh in range(2):
                            ps = ppool.tile([128, 512], dt.float32, tag="ps", name="ps")
                            nc.tensor.matmul(
                                ps[:],
                                lhsT[:],
                                proj_sb[:, nh * 512 : (nh + 1) * 512],
                                start=True,
                                stop=True,
                            )
                            dst = out_sb[:, ti, nh * 512 : (nh + 1) * 512]
                            if ceng is nc.vector:
                                ceng.tensor_copy(dst, ps[:])
                            else:
                                ceng.copy(dst, ps[:])
                    t0 = p["obase"] + tb
                    has_partial = (tb + gsz == nt) and nrow_last < 128
                    nfull = gsz - 1 if has_partial else gsz
                    if nfull:
                        nc.sync.dma_start(
                            outmm_par[:, t0 : t0 + nfull, :], out_sb[:, :nfull, :]
                        )
                    if has_partial:
                        nc.sync.dma_start(
                            outmm_par[:nrow_last, t0 + nfull, :],
                            out_sb[:nrow_last, nfull, :],
                        )

    nc.compile()
    return nc


def _host_tables(emb0, emb1, emb2, emb3, proj0, proj1, proj2, proj3):
    # fold embedding + projection of buckets 0/1 into one table (f32
    # accumulate, then bf16)
    pp0 = (emb0 @ proj0.T) * EMB_SCALE
    pp1 = (emb1 @ proj1.T) * EMB_SCALE
    pp01 = np.ascontiguousarray(
        np.concatenate([pp0, pp1], axis=0).astype(BF16)
    )
    # merged bucket-2/3 table: e2 rows in cols 0:64, e3 rows in cols 64:80
    em23 = np.zeros((N_TOKEN - 40000, D23), BF16)
    em23[: emb2.shape[0], :64] = emb2.astype(BF16)
    em23[emb2.shape[0] :, 64:] = emb3.astype(BF16)
    # stacked projection [80, 1024]
    p23 = np.zeros((D23, D_PROJ), np.float32)
    p23[:64] = proj2.T * EMB_SCALE
    p23[64:] = proj3.T * EMB_SCALE
    return pp01, np.ascontiguousarray(em23), np.ascontiguousarray(p23.astype(BF16))


def kernel(inp, emb0, emb1, emb2, emb3, proj0, proj1, proj2, proj3):
    global LAST_RESULT
    ids = np.asarray(inp).reshape(-1).astype(np.int64)
    n_tok = ids.shape[0]
    assert n_tok % NCORES == 0

    pp01, em23, p23 = _host_tables(
        np.asarray(emb0), np.asarray(emb1), np.asarray(emb2), np.asarray(emb3),
        np.asarray(proj0), np.asarray(proj1), np.asarray(proj2), np.asarray(proj3),
    )
    tables = {"pp01": pp01, "em23": em23}
    ident_host = np.eye(128, dtype=BF16)

    # --- sort + segment + deal round-robin to cores ---
    order = np.argsort(ids, kind="stable")
    sids = ids[order]

    plan = []  # per active segment: si, nt, n_live, cb, obase
    core_idx = [[] for _ in range(NCORES)]
    unshard = []  # (out tensor name, tile base, [token positions per core])
    cb = 0
    obase = {"mm": 0, "direct": 0}
    for si in SEG_ORDER:
        seg = SEGS[si]
        g_lo = np.searchsorted(sids, seg["lo"], "left")
        g_hi = np.searchsorted(sids, seg["hi"], "left")
        if g_hi <= g_lo:
            continue
        toks = order[g_lo:g_hi]
        locs = (sids[g_lo:g_hi] - seg["lo"]).astype(np.int32)
        counts = [len(locs[c::NCORES]) for c in range(NCORES)]
        n_live = max(counts)
        n_pad = -(-n_live // 128) * 128
        nt = n_pad // 128
        per_core_toks = []
        for c in range(NCORES):
            li = locs[c::NCORES]
            pad = np.zeros(n_pad, np.int32)
            pad[: len(li)] = li
            # idx col cb+t, partition p holds the row for slot t*128+p
            core_idx[c].append(pad.reshape(nt, 128).T)
            per_core_toks.append(toks[c::NCORES])
        kind = seg["kind"]
        plan.append(
            {"si": si, "nt": nt, "n_live": n_live, "cb": cb, "obase": obase[kind]}
        )
        unshard.append(
            ("outmm" if kind == "mm" else "outpp", obase[kind], per_core_toks)
        )
        cb += nt
        obase[kind] += nt
    nt_total = cb

    in_maps = []
    for c in range(NCORES):
        idx_host = np.ascontiguousarray(np.concatenate(core_idx[c], axis=1))
        m = {"ident": ident_host, "idxs": idx_host, "projt": p23}
        for p in plan:
            s = SEGS[p["si"]]
            m[s["name"]] = tables[s["name"]]
        in_maps.append(m)

    nc = _build_graph(plan, nt_total, 0)
    res = run_bass_kernel_spmd(nc, in_maps, core_ids=list(range(NCORES)))
    LAST_RESULT = res

    # --- unshard: undo the sort permutation, widen to f32 ---
    full = np.empty((n_tok, D_PROJ), np.float32)
    for c in range(NCORES):
        rows_by = {
            name: np.asarray(res.results[c][name])
            .transpose(1, 0, 2)
            .reshape(-1, D_PROJ)
            for name in ("outmm", "outpp")
        }
        for (name, tb, per_core_toks) in unshard:
            toks = per_core_toks[c]
            if len(toks):
                full[toks] = rows_by[name][tb * 128 : tb * 128 + len(toks)]
    B, S = np.asarray(inp).shape
    return full.reshape(B, S, D_PROJ)
